# revision 41
# baseline (speedup 1.0000x reference)
"""Trainium2 Bass kernel for nn_KnowledgeAttention.

Math (per batch example b):
    sim[k]  = cos_sim(pooled[b], kg_key[b,k])                      # [K]
    q       = (hs @ Wq.T + bq) * HD**-0.5     -> heads [T,H,HD]
    k       = kg_value @ Wk.T + bk            -> heads [K,H,HD]
    v       = kg_value @ Wv.T + bv            -> heads [K,H,HD]
    S[h,t,k]= q_h[t]·k_h[k] + beta[h]*sim[k]
    P       = softmax_k(S);  O[t,h] = sum_k P v
    out     = O @ Wo.T + bo
Sharding: pure data-parallel over batch — 8 examples on 8 cores, weights
replicated, no collectives.

Per-core strategy (v3):
  * hs.T and kg_value.T pre-transposed/pre-cast to bf16 on the host.
  * The per-head cosine bias beta_h*sim[k] rides the score exp as the ACT
    per-partition bias operand: e = exp(S.T[k,t] + bias[k]). No w-folding
    into V, so the V tiles depend only on kg_value/Wv (startup decoupled
    from the cosine-sim phase).
  * Scores are computed transposed S.T[k,t] with 64-deep contraction;
    the even/odd head matmuls of one k-chunk write the two halves of ONE
    2-bank psum tile, so both become ready together and run concurrently
    in the PE array via (64,128) tiling at positions (0,0)/(64,0).
  * The softmax denominator rides the AV matmul: AV lhsT per head pair j
    is [v_even | ones | ones | v_odd] (one [128,1536] tile per k-chunk),
    so psum rows 64:128 (even) / 0:64 (odd) hold the denominator.
  * Inputs stream on three DMA queues (sync/scalar/gpsimd) ordered by
    first use: kproj -> vproj -> qproj(win0) -> attention loop, with the
    small cosine-phase inputs on their own queue so their completion
    isn't gated behind the big weight loads.
  * q-projection / attention / out-projection are pipelined per 512-wide
    t-window.
"""

import sys

import numpy as np

# ---------------------------------------------------------------- constants
BS = 8
T = 2048
D = 768
H = 12
HD = 64
K = 512
SCALE = HD ** -0.5
EPS = 1e-8
DC = D // 128   # 6 contraction/partition chunks of 128 over D
KC = K // 128   # 4 chunks over K
TW = 512        # t window for moving operand
NTW = T // TW   # 4
NPAIR = H // 2  # 6 head pairs

TRACE = False
LAST_EXEC_NS = None

_CACHE = {}


def _ensure_path():
    try:
        import concourse  # noqa: F401
    except ImportError:
        for p in ("/opt/trn_rl_repo", "/root/.axon_site/_ro/trn_rl_repo"):
            if p not in sys.path:
                sys.path.insert(0, p)


def _build_program():
    _ensure_path()
    import concourse.bass as bass
    import concourse.mybir as mybir
    import concourse.tile as tile
    from concourse import bacc
    from contextlib import ExitStack

    F32 = mybir.dt.float32
    BF16 = mybir.dt.bfloat16
    Alu = mybir.AluOpType
    Act = mybir.ActivationFunctionType

    nc = bacc.Bacc("TRN2", target_bir_lowering=False, debug=False, num_devices=BS)

    # packed inputs: one DMA instruction each (sync-queue issue is ~0.6us per
    # dma_start, so instruction count dominates startup latency)
    hst_d = nc.dram_tensor("hstp", [128, DC * T], BF16, kind="ExternalInput").ap()
    kgvt_d = nc.dram_tensor("kgvtp", [128, DC * K], BF16, kind="ExternalInput").ap()
    kgk_d = nc.dram_tensor("kgkp", [128, KC * D], F32, kind="ExternalInput").ap()
    wqt_d = nc.dram_tensor("wqp", [128, DC * D], BF16, kind="ExternalInput").ap()
    wkt_d = nc.dram_tensor("wkp", [128, DC * D], BF16, kind="ExternalInput").ap()
    wvt_d = nc.dram_tensor("wvp", [128, DC * D], BF16, kind="ExternalInput").ap()
    wot_d = nc.dram_tensor("wop", [128, DC * D], BF16, kind="ExternalInput").ap()
    row_d = nc.dram_tensor("rowp", [1, 2 * D + H], F32, kind="ExternalInput").ap()
    pb_d = nc.dram_tensor("pbp", [128, 2 * DC], F32, kind="ExternalInput").ap()
    keep_d = nc.dram_tensor("keepo", [1, 8], F32, kind="ExternalOutput").ap()
    out_d = nc.dram_tensor("out", [T, D], F32, kind="ExternalOutput").ap()

    with tile.TileContext(nc) as tc, ExitStack() as ctx:
        const = ctx.enter_context(tc.tile_pool(name="const", bufs=1))
        inp = ctx.enter_context(tc.tile_pool(name="inp", bufs=4))
        big = ctx.enter_context(tc.tile_pool(name="big", bufs=12))
        kt_p = ctx.enter_context(tc.tile_pool(name="ktp", bufs=6))
        v_p = ctx.enter_context(tc.tile_pool(name="vp", bufs=4))
        e_p = ctx.enter_context(tc.tile_pool(name="ep", bufs=8))
        r_p = ctx.enter_context(tc.tile_pool(name="rp", bufs=4))
        fin_p = ctx.enter_context(tc.tile_pool(name="finp", bufs=2))
        sm_p = ctx.enter_context(tc.tile_pool(name="smp", bufs=4))
        ps = ctx.enter_context(tc.tile_pool(name="ps", bufs=2, space="PSUM"))

        warmsrc = const.tile([128, TW], BF16, tag="warmsrc")
        nc.vector.memset(warmsrc[:], 0.001)

        # ---------------- DMA front: ONE queue, strict need order ----------
        # A single queue gives strict delivery priority: the DMA engines
        # stream each tensor at full aggregate bandwidth before starting the
        # next, so kproj/vproj/qproj inputs land as early as possible.
        kgvt = const.tile([128, DC * K], BF16, tag="kgvt")
        nc.sync.dma_start(kgvt[:], kgvt_d)
        # wk is packed m-major (output-chunk major) and split in two DMAs so
        # the first kproj chains can start as soon as the first half lands
        wk_sb = const.tile([128, DC * D], BF16, tag="wk")
        nc.sync.dma_start(wk_sb[:, 0:3 * D], wkt_d[:, 0:3 * D])
        nc.sync.dma_start(wk_sb[:, 3 * D:DC * D], wkt_d[:, 3 * D:DC * D])
        # tiny tensors next: they unblock the kproj bias-adds and the
        # cosine-phase broadcasts at negligible delay to the big loads
        rowv = const.tile([1, 2 * D + H], F32, tag="rowv")
        nc.sync.dma_start(rowv[:], row_d)
        pb_sb = const.tile([128, 2 * DC], F32, tag="pb_sb")
        nc.sync.dma_start(pb_sb[:], pb_d)
        wv_sb = const.tile([128, DC * D], BF16, tag="wv")
        nc.sync.dma_start(wv_sb[:], wvt_d)
        hst = const.tile([128, DC * T], BF16, tag="hst")
        nc.sync.dma_start(
            hst[:].rearrange("p (c t) -> p c t", c=DC)[:, :, 0:TW],
            hst_d.rearrange("p (c t) -> p c t", c=DC)[:, :, 0:TW])
        kgk_sb = const.tile([128, KC * D], F32, tag="kgk_sb")
        nc.sync.dma_start(kgk_sb[:], kgk_d)
        wq_sb = const.tile([128, DC * D], BF16, tag="wq")
        nc.sync.dma_start(wq_sb[:], wqt_d)
        wo_sb = const.tile([128, DC * D], BF16, tag="wo")
        nc.sync.dma_start(wo_sb[:], wot_d)
        nc.sync.dma_start(
            hst[:].rearrange("p (c t) -> p c t", c=DC)[:, :, TW:T],
            hst_d.rearrange("p (c t) -> p c t", c=DC)[:, :, TW:T])

        # PE warmup: keep HAM at K=8/8 through the DMA-wait window. Result is
        # exported via the tiny keep output so nothing downstream waits on it.
        warm_ps = ps.tile([128, TW], F32, tag="o", bufs=2, name="warm")
        for wi in range(16):
            nc.tensor.matmul(
                warm_ps[:], warmsrc[:, 0:128], warmsrc[:],
                start=(wi == 0), stop=(wi == 15))
        keep_sb = const.tile([1, 8], F32, tag="keep_sb")
        nc.vector.tensor_copy(keep_sb[:, 0:4], warm_ps[0:1, 0:4])

        pl = rowv[0:1, 0:D]
        bo_row = rowv[0:1, D:2 * D]
        bt = rowv[0:1, 2 * D:2 * D + H]

        bo_bc = const.tile([128, D], F32, tag="bo_bc")
        nc.gpsimd.partition_broadcast(bo_bc[:], bo_row, channels=128)
        beta_bc = const.tile([128, H], F32, tag="beta_bc")
        nc.gpsimd.partition_broadcast(beta_bc[:], bt, channels=128)
        pl_bc = const.tile([128, D], F32, tag="pl_bc")
        nc.gpsimd.partition_broadcast(pl_bc[:], pl, channels=128)

        # -------- phase 0: bias_all[k_part, kc*H+h] = beta_h * sim[k] --------
        # norms^2 batched into one [128,5] tile -> single ACT sqrt
        nrm2 = sm_p.tile([128, 5], F32, tag="nrm2")
        dots = sm_p.tile([128, 4], F32, tag="dots")
        sq = inp.tile([128, D], F32, tag="inp", name="sq")
        nc.vector.scalar_tensor_tensor(
            out=sq[:], in0=pl_bc[:], scalar=1.0, in1=pl_bc[:],
            op0=Alu.mult, op1=Alu.mult, accum_out=nrm2[:, 0:1])
        for c in range(KC):
            kk = kgk_sb[:, c * D:(c + 1) * D]
            sq2 = inp.tile([128, D], F32, tag="inp")
            nc.vector.scalar_tensor_tensor(
                out=sq2[:], in0=kk, scalar=1.0, in1=kk,
                op0=Alu.mult, op1=Alu.mult, accum_out=nrm2[:, 1 + c:2 + c])
            sq3 = inp.tile([128, D], F32, tag="inp")
            nc.vector.scalar_tensor_tensor(
                out=sq3[:], in0=kk, scalar=1.0, in1=pl_bc[:],
                op0=Alu.mult, op1=Alu.mult, accum_out=dots[:, c:c + 1])
        nrms = sm_p.tile([128, 5], F32, tag="nrms")
        nc.scalar.activation(nrms[:], nrm2[:], Act.Sqrt)
        nc.vector.tensor_scalar_max(nrms[:], nrms[:], EPS)
        rcp = sm_p.tile([128, 5], F32, tag="rcp")
        nc.vector.reciprocal(rcp[:], nrms[:])
        # chain an exp off the sqrt result so the ACT exp table loads during
        # the idle startup window, not in front of the first score exp
        nc.scalar.activation(keep_sb[:, 4:8], nrms[0:1, 0:4], Act.Exp)
        nc.sync.dma_start(keep_d, keep_sb[:])
        nc.vector.tensor_mul(dots[:], dots[:], rcp[:, 1:5])
        nc.vector.tensor_scalar_mul(dots[:], dots[:], rcp[:, 0:1])
        bias_all = const.tile([128, KC * H], F32, tag="bias_all")
        for c in range(KC):
            nc.vector.tensor_scalar_mul(
                bias_all[:, c * H:(c + 1) * H], beta_bc[:], dots[:, c:c + 1])

        # ---------------- phase 1a: k.T tiles --------------------------------
        kt = [kt_p.tile([128, K], BF16, tag="kt", name="kt") for _ in range(DC)]
        for m in range(DC):
            pk = ps.tile([128, K], F32, tag="mm", bufs=2)
            for c in range(DC):
                nc.tensor.matmul(
                    pk[:], wk_sb[:, m * D + c * 128:m * D + (c + 1) * 128],
                    kgvt[:, c * K:(c + 1) * K],
                    start=(c == 0), stop=(c == DC - 1))
            nc.scalar.activation(kt[m][:], pk[:], Act.Identity,
                                 bias=pb_sb[:, DC + m:DC + m + 1])

        # ------- phase 1b: AV stationary tiles [v_2j | ones | ones | v_2j+1]
        # vEO[kc][:, j*256+off]: off 0:64 v_even, 64:192 ones, 192:256 v_odd
        vEO = [v_p.tile([128, NPAIR * 256], BF16, tag="vEO", name="vEO")
               for _ in range(KC)]
        for kc in range(KC):
            nc.vector.memset(
                vEO[kc][:].rearrange("p (j b) -> p j b", b=256)[:, :, 64:192], 1.0)
        for n in range(2):
            for kc in range(KC):
                pv = ps.tile([128, 384], F32, tag="mm", bufs=2)
                for c in range(DC):
                    nc.tensor.matmul(
                        pv[:], kgvt[:, c * K + kc * 128:c * K + (kc + 1) * 128],
                        wv_sb[:, c * D + n * 384:c * D + (n + 1) * 384],
                        start=(c == 0), stop=(c == DC - 1))
                src = pv[:].rearrange("p (a two b) -> p a two b", two=2, b=64)
                dst = (vEO[kc][:, 3 * n * 256:(3 * n + 3) * 256]
                       .rearrange("p (a b) -> p a b", b=256))
                nc.scalar.activation(dst[:, :, 0:64], src[:, :, 0, :], Act.Copy)
                nc.scalar.activation(dst[:, :, 192:256], src[:, :, 1, :], Act.Copy)

        qt = [big.tile([128, T], BF16, tag="big", name="qt") for _ in range(DC)]
        ot = [big.tile([128, T], BF16, tag="big", name="ot") for _ in range(NPAIR)]

        # ------- per t-window: q-proj / attention / out-proj interleaved -------
        # window widths taper at the end so the trailing out-projection after
        # the last attention window is short
        WINS = [512, 512, 512, 512]
        OFFS = [0, 512, 1024, 1536]
        NW = len(WINS)

        def qproj_chunk(off, w, m):
            pq = ps.tile([128, w], F32, tag="mm", bufs=2)
            for c in range(DC):
                nc.tensor.matmul(
                    pq[:], wq_sb[:, c * D + m * 128:c * D + (m + 1) * 128],
                    hst[:, c * T + off:c * T + off + w],
                    start=(c == 0), stop=(c == DC - 1))
            nc.vector.tensor_scalar_add(
                qt[m][:, off:off + w], pq[:], pb_sb[:, m:m + 1])

        def oproj_tsub(tc16):
            fin = fin_p.tile([128, D], F32, tag="fin")
            for n in range(2):
                pf = ps.tile([128, 384], F32, tag="mm", bufs=2)
                for c in range(DC):
                    nc.tensor.matmul(
                        pf[:], ot[c][:, tc16 * 128:(tc16 + 1) * 128],
                        wo_sb[:, c * D + n * 384:c * D + (n + 1) * 384],
                        start=(c == 0), stop=(c == DC - 1))
                nc.vector.tensor_add(
                    fin[:, n * 384:(n + 1) * 384], pf[:],
                    bo_bc[:, n * 384:(n + 1) * 384])
            nc.sync.dma_start(out_d[tc16 * 128:(tc16 + 1) * 128, :], fin[:])

        for m in range(DC):
            qproj_chunk(OFFS[0], WINS[0], m)

        def av_block(j, off, w):
            # AV + normalize for head pair j of the window at (off, w); runs
            # one slot behind scores/exp so its matmuls never wait on the
            # exps and stream back-to-back.
            tw = slice(off, off + w)
            eP = eP_save[j % 2]
            poE = ps.tile([128, w], F32, tag="o", bufs=2, name="poE")
            poO = ps.tile([128, w], F32, tag="o", bufs=2, name="poO")
            for kc in range(KC):
                nc.tensor.matmul(
                    poE[:], vEO[kc][:, j * 256:j * 256 + 128],
                    eP[kc][:, 0:w],
                    start=(kc == 0), stop=(kc == KC - 1))
            for kc in range(KC):
                nc.tensor.matmul(
                    poO[:], vEO[kc][:, j * 256 + 128:(j + 1) * 256],
                    eP[kc][:, w:2 * w],
                    start=(kc == 0), stop=(kc == KC - 1))
            # full-128 recip (base-0): garbage on the data rows is unread
            rallE = r_p.tile([128, w], F32, tag="rall", name="rallE")
            rallO = r_p.tile([128, w], F32, tag="rall", name="rallO")
            nc.vector.reciprocal_approx_fast(rallE[:], poE[:])
            nc.vector.reciprocal_approx_fast(rallO[:], poO[:])
            nc.vector.tensor_mul(
                ot[j][0:64, tw], poE[0:64, :], rallE[64:128, :])
            nc.vector.tensor_mul(
                ot[j][64:128, tw], poO[64:128, :], rallO[0:64, :])

        eP_save = [None, None]
        prev = None  # (j, off, w) of the slot whose AV is pending
        for wi in range(NW):
            off, w = OFFS[wi], WINS[wi]
            tw = slice(off, off + w)
            for j in range(NPAIR):
                # scores + exp: each kc's even/odd head matmuls fill one
                # psum tile so they pair in the PE array; the cosine bias
                # rides the exp as a per-partition ACT bias. The O-half
                # always starts at column 512 (a PSUM bank boundary) so the
                # paired matmuls never write the same bank concurrently.
                eP = []
                for kc in range(KC):
                    sP = ps.tile([128, 2 * TW], F32, tag="s", bufs=2, name="sP")
                    nc.tensor.matmul(
                        sP[:, 0:w],
                        kt[j][0:64, kc * 128:(kc + 1) * 128],
                        qt[j][0:64, tw], start=True, stop=True)
                    nc.tensor.matmul(
                        sP[:, TW:TW + w],
                        kt[j][64:128, kc * 128:(kc + 1) * 128],
                        qt[j][64:128, tw], start=True, stop=True)
                    e = e_p.tile([128, 2 * w], BF16, tag="e")
                    nc.scalar.activation(
                        e[:, 0:w], sP[:, 0:w], Act.Exp,
                        bias=bias_all[:, kc * H + 2 * j:kc * H + 2 * j + 1])
                    nc.scalar.activation(
                        e[:, w:2 * w], sP[:, TW:TW + w], Act.Exp,
                        bias=bias_all[:, kc * H + 2 * j + 1:kc * H + 2 * j + 2])
                    eP.append(e)
                eP_save[j % 2] = eP

                if prev is not None:
                    av_block(*prev)
                prev = (j, off, w)

                # fill PE exp-wait gaps with projection work (oproj shifted
                # by one j because ot[w][j=5] lands in slot (w+1, j=0))
                if wi > 0 and 1 <= j < WINS[wi - 1] // 128 + 1:
                    oproj_tsub(OFFS[wi - 1] // 128 + j - 1)
                if wi < NW - 1:
                    qproj_chunk(OFFS[wi + 1], WINS[wi + 1], j)

        av_block(*prev)
        for tsub in range(WINS[-1] // 128):
            oproj_tsub(OFFS[-1] // 128 + tsub)

    nc.compile()
    return nc


def _get_program():
    if "nc" not in _CACHE:
        _CACHE["nc"] = _build_program()
    return _CACHE["nc"]


def _host_prep(inputs):
    import ml_dtypes
    bf16 = ml_dtypes.bfloat16

    f32 = lambda x: np.ascontiguousarray(np.asarray(x, dtype=np.float32))
    Wq, Wk, Wv, Wo = (f32(inputs[k]) for k in ("Wq", "Wk", "Wv", "Wo"))
    bq, bk, bv, bo = (f32(inputs[k]) for k in ("bq", "bk", "bv", "bo"))
    beta = f32(inputs["beta"])

    def pack(a, dtype):
        # [C*128, X] -> [128, C*X] with [:, c*X+x] = a[c*128+p, x]
        C = a.shape[0] // 128
        return np.ascontiguousarray(
            a.reshape(C, 128, -1).transpose(1, 0, 2).reshape(128, -1)
            .astype(dtype))

    bo_eff = (bo + bv @ Wo.T).astype(np.float32)
    rowp = np.zeros((1, 2 * D + H), np.float32)
    pooled_all = f32(inputs["pooled_hidden_states"])
    rowp[0, D:2 * D] = bo_eff
    rowp[0, 2 * D:] = beta
    pbp = np.zeros((128, 2 * DC), np.float32)
    pbp[:, 0:DC] = (bq * SCALE).reshape(DC, 128).T
    pbp[:, DC:] = bk.reshape(DC, 128).T

    def pack_mmaj(a, dtype):
        # [768, 768] -> [128, m*768 + c*128 + x] = a[c*128+p, m*128+x]
        a4 = a.reshape(DC, 128, DC, 128)
        return np.ascontiguousarray(
            a4.transpose(1, 2, 0, 3).reshape(128, DC * D).astype(dtype))

    shared = {
        "wqp": pack(np.ascontiguousarray(Wq.T * SCALE), bf16),
        "wkp": pack_mmaj(np.ascontiguousarray(Wk.T), bf16),
        "wvp": pack(np.ascontiguousarray(Wv.T), bf16),
        "wop": pack(np.ascontiguousarray(Wo.T), bf16),
        "pbp": pbp,
    }

    hs = f32(inputs["hidden_states"])
    kgk = f32(inputs["kg_key"])
    kgv = f32(inputs["kg_value"])

    in_maps = []
    for b in range(BS):
        m = dict(shared)
        m["hstp"] = pack(np.ascontiguousarray(hs[b].T), bf16)
        m["kgvtp"] = pack(np.ascontiguousarray(kgv[b].T), bf16)
        m["kgkp"] = pack(kgk[b], np.float32)
        rb = rowp.copy()
        rb[0, 0:D] = pooled_all[b]
        m["rowp"] = rb
        in_maps.append(m)
    return in_maps




def _install_ntff_hook():
    """Register the axon NTFF profile hook so trace=True yields exec_time_ns.

    Only used from our own test harness (TRACE=True); the default kernel()
    path never calls this.
    """
    try:
        from antenv.axon_hooks import get_axon_ntff_profile_hook  # noqa: F401
        return
    except ImportError:
        pass
    import contextlib
    import ctypes
    import types

    so_path = "/opt/axon/libaxon_pjrt.so"
    try:
        lib = ctypes.CDLL(so_path)
    except OSError:
        return
    if not hasattr(lib, "axon_start_nrt_profile"):
        return
    lib.axon_start_nrt_profile.argtypes = [
        ctypes.POINTER(ctypes.c_int64), ctypes.c_size_t]
    lib.axon_start_nrt_profile.restype = ctypes.c_int64
    lib.axon_stop_nrt_profile.argtypes = [ctypes.c_char_p]
    lib.axon_stop_nrt_profile.restype = ctypes.c_int64

    @contextlib.contextmanager
    def _hook(output_dir, device_ids):
        import jax
        jax.devices()
        if device_ids:
            ids = (ctypes.c_int64 * len(device_ids))(*device_ids)
            rc = lib.axon_start_nrt_profile(ids, len(device_ids))
        else:
            rc = lib.axon_start_nrt_profile(None, 0)
        if rc != 0:
            raise RuntimeError(f"axon_start_nrt_profile rc={rc}")
        try:
            yield
        finally:
            n = lib.axon_stop_nrt_profile(str(output_dir).encode())
            print(f"profile: {n} file(s) written to {output_dir}",
                  file=sys.stderr)

    mod = types.ModuleType("antenv.axon_hooks")
    mod.get_axon_ntff_profile_hook = lambda: _hook
    mod.set_axon_ntff_profile_hook = lambda h: None
    sys.modules["antenv.axon_hooks"] = mod


def kernel(**inputs):
    global LAST_EXEC_NS
    _ensure_path()
    from concourse import bass_utils

    if TRACE:
        _install_ntff_hook()
    nc = _get_program()
    in_maps = _host_prep(inputs)
    last_err = None
    for attempt in range(3):
        try:
            res = bass_utils.run_bass_kernel_spmd(
                nc, in_maps, core_ids=list(range(BS)), trace=TRACE)
            break
        except Exception as e:  # transient device/runtime hiccup: retry
            last_err = e
            if attempt == 2:
                raise
            print(f"kernel run attempt {attempt} failed ({e}); retrying",
                  file=sys.stderr)
    LAST_EXEC_NS = res.exec_time_ns
    out = np.stack([res.results[b]["out"] for b in range(BS)], axis=0)
    return out.astype(np.float32)
